# revision 23
# baseline (speedup 1.0000x reference)
"""CondGraphConv Trainium2 kernel: 8-core SPMD, j-sharded edges. v2

Algebra:
    x_e   = Ci[i_e] + Cj[j_e] + s_e @ Wls          (s_e = relu(sp_e@Ws+bs), host-precomputed)
    out_e = relu(LN(x_e) * gamma[bid[j_e]] + beta[bid[j_e]])
  with Ci = h @ Wl[:128], Cj = h @ Wl[128:256], h = relu(nf @ Wn + bn).

Sharding: core c owns edges with j in [800c, 800c+800), sorted by j.
Phase 1 is node-sharded: each core projects its own 800 nodes, then an
AllGather builds the full Ci table [6400,128] f16 in DRAM.  Per-window
tables Wwin[w] = [[Cj|gamma|beta] rows for 98 nodes ; [Wls|0|0] rows]
stay resident in SBUF.

Phase 2 per 128-edge tile (edges sorted by j, tiles within 98-node
windows):
  - ONE matmul vs Wwin: lhsT = [one-hot(98 rows) ; sT(30 rows)] baked on
    host, DMA'd partition-major -> PSUM [slot, 0:384] = [x_cj+x_s|g|b]
  - Ci rows arrive via gpsimd.dma_gather (2048 rows/instr, 5-deep
    prefetch) and accumulate via ONE batched identity matmul per
    4 tiles
  - LN: bn_stats batched 4 tiles/instr + manual even/odd combine; rstd
    via scalar Rsqrt; xn via scalar activation (per-partition scale/bias)
  - FiLM from PSUM + relu; f16 output written partition-major, host
    inverse-permutes/upcasts.
"""

import sys
import types

for _p in ("/opt/trn_rl_repo",):
    if _p not in sys.path:
        sys.path.append(_p)

import numpy as np

N, E, B = 6400, 313600, 128
NODE_DIM, COND_DIM, EDGE_DIM = 2048, 1024, 128
S_IN, S_OUT = 8, 30
EPS = 1e-5

NCORES = 8
NLOC = N // NCORES            # 800 own nodes per core
WROWS = 98                    # one-hot rows per lhsT tile (98 + 30 = 128)
NWIN = (NLOC + WROWS - 1) // WROWS   # 9 local windows
TILE = 128
TPC = 16                      # tiles per staging chunk
TPG = 4                       # tiles per PSUM group
GCH = 16                      # tiles per indirect gather (2048 rows)
F16 = np.float16

_cache = {}


def _axon_shim():
    try:
        import antenv.axon_hooks  # noqa: F401
        return
    except ImportError:
        pass
    try:
        import antenv
        from trn_agent_boot.trn_boot import _ntff_profile_via_ctypes
    except ImportError:
        return
    mod = types.ModuleType("antenv.axon_hooks")
    holder = [None]
    mod.set_axon_ntff_profile_hook = lambda h: holder.__setitem__(0, h)
    mod.get_axon_ntff_profile_hook = lambda: holder[0]
    sys.modules["antenv.axon_hooks"] = mod
    antenv.axon_hooks = mod
    try:
        mod.set_axon_ntff_profile_hook(
            _ntff_profile_via_ctypes("/opt/axon/libaxon_pjrt.so")
        )
    except Exception:
        pass


def _plan(inputs):
    """Shard edges by j-range, sort by j, tile within 98-node windows,
    build the core-uniform window schedule."""
    jj = np.asarray(inputs["node_j_ids"]).astype(np.int64)

    plans = []
    counts = np.zeros((NCORES, NWIN), np.int64)
    for c in range(NCORES):
        lo = c * NLOC
        eids = np.nonzero((jj >= lo) & (jj < lo + NLOC))[0]
        order = np.argsort(jj[eids], kind="stable")
        eids = eids[order]
        jl = jj[eids] - lo
        wb = jl // WROWS
        tiles = []
        s = 0
        ne = eids.shape[0]
        while s < ne:
            b = wb[s]
            e = min(s + TILE, ne)
            e = s + int(np.searchsorted(wb[s:e], b + 1))
            tiles.append((s, e - s, int(b)))
            counts[c, int(b)] += 1
            s = e
        plans.append({"eids": eids, "jl": jl, "tiles": tiles})

    maxcnt = counts.max(axis=0)
    sched = []
    for b in range(NWIN):
        sched.extend([b] * int(maxcnt[b]))
    while len(sched) % GCH:
        sched.append(0)
    return plans, sched


def _prep_inputs(inputs, plans, sched):
    nt = len(sched)
    ngc = nt // GCH
    KC_GB = COND_DIM // 128 + 1
    KDIM_GB = KC_GB * 128

    ii = np.asarray(inputs["node_i_ids"]).astype(np.int64)
    nf = np.asarray(inputs["node_feats"], np.float32)
    wnA = np.asarray(inputs["Wn"], np.float32).astype(F16)
    bnc = np.asarray(inputs["bn"], np.float32).reshape(128, 1)

    cond = np.asarray(inputs["cond_feats"], np.float32)
    condA = np.zeros((KDIM_GB, B), F16)
    condA[:COND_DIM] = cond.T.astype(F16)
    condA[COND_DIM] = 1.0
    wcA = np.zeros((KDIM_GB, 256), F16)
    wcA[:COND_DIM] = np.asarray(inputs["Wc"], np.float32).astype(F16)
    bc_plus = np.asarray(inputs["bc"], np.float32).copy()
    bc_plus[:EDGE_DIM] += 1.0          # gamma = 1 + (...)
    wcA[COND_DIM] = bc_plus.astype(F16)

    wl = np.asarray(inputs["Wl"], np.float32)
    wli = wl[:128].astype(F16).copy()
    wlj = wl[128:256].astype(F16).copy()
    wls = wl[256:].astype(F16).copy()

    # host precompute: s = relu(sp @ Ws + bs)  [E, 30]
    sp = np.asarray(inputs["spatial_feats"], np.float32)
    ws = np.asarray(inputs["Ws"], np.float32)
    bss = np.asarray(inputs["bs"], np.float32)
    s_full = np.maximum(sp @ ws + bss, 0.0).astype(F16)      # [E, 30]

    bid = np.asarray(inputs["batch_ids"]).astype(np.int64)
    idn = np.eye(128, dtype=F16)

    shared = dict(wnA=wnA, condA=condA, wcA=wcA, bnc=bnc,
                  wli=wli, wlj=wlj, wls=wls, idn=idn)

    in_maps = []
    for c, p in enumerate(plans):
        nfT_c = np.ascontiguousarray(
            nf[c * NLOC:(c + 1) * NLOC].T.astype(F16))      # [2048, 800]
        # per-window one-hot selecting each node's batch row
        ohb = np.zeros((NWIN, 128, WROWS), F16)             # [blk, batch, node]
        for blk in range(NWIN):
            g0 = c * NLOC + blk * WROWS
            w = min(WROWS, NLOC - blk * WROWS)
            nb = bid[g0:g0 + w]
            ohb[blk, nb, np.arange(w)] = 1.0

        # distribute this core's tiles into the uniform schedule slots
        slot_of_block = {}
        for t, b in enumerate(sched):
            slot_of_block.setdefault(b, []).append(t)
        used = {b: 0 for b in range(NWIN)}
        # lhsT chunks: rows 0:98 one-hot over window nodes, 98:128 sT
        A = np.zeros((128, nt, TILE), F16)                  # [row, t, slot]
        idxI = np.zeros((128, nt), np.int32)                # [slot, t] -> i id
        slotmap = np.full((128, nt), -1, np.int64)          # [slot, t] -> edge
        for (s0, cnt, b) in p["tiles"]:
            t = slot_of_block[b][used[b]]
            used[b] += 1
            rows = p["jl"][s0:s0 + cnt] - b * WROWS
            A[rows, t, np.arange(cnt)] = 1.0
            e_ids = p["eids"][s0:s0 + cnt]
            A[WROWS:WROWS + S_OUT, t, 0:cnt] = s_full[e_ids].T
            idxI[0:cnt, t] = ii[e_ids].astype(np.int32)
            slotmap[0:cnt, t] = e_ids
        # dma_gather idx layout: flat k = t_local*128 + slot per chunk;
        # hw wrap [16, 256] col-major tiled x8 core groups
        idxT = idxI.T.reshape(ngc, GCH * TILE)              # [g, k]
        idxW = np.zeros((ngc, 128, GCH * TILE // 16), np.int16)
        for g in range(ngc):
            w16 = np.ascontiguousarray(
                idxT[g].reshape(-1, 16).T)                  # [16, 256]
            idxW[g] = np.tile(w16, (8, 1))
        m = dict(shared)
        m["nfT"] = nfT_c
        m["ohb"] = ohb
        m["A"] = A
        m["idxW"] = idxW
        in_maps.append(m)
        p["slotmap"] = slotmap
    return in_maps


def _build_program(sched):
    import concourse.bass as bass  # noqa: F401
    import concourse.tile as tile
    from concourse import bacc, mybir
    from contextlib import ExitStack

    f16 = mybir.dt.float16
    f32 = mybir.dt.float32
    i16 = mybir.dt.int16
    AF = mybir.ActivationFunctionType
    OP = mybir.AluOpType

    KC_H = NODE_DIM // 128       # 16
    KC_GB = COND_DIM // 128 + 1  # 9
    nt = len(sched)
    ngc = nt // GCH

    nc = bacc.Bacc(
        "TRN2", target_bir_lowering=False, debug=False,
        num_devices=NCORES, num_swdge_queues=1,
    )

    nfT = nc.dram_tensor("nfT", [NODE_DIM, NLOC], f16, kind="ExternalInput")
    wnA = nc.dram_tensor("wnA", [NODE_DIM, 128], f16, kind="ExternalInput")
    condA = nc.dram_tensor("condA", [KC_GB * 128, B], f16, kind="ExternalInput")
    wcA = nc.dram_tensor("wcA", [KC_GB * 128, 256], f16, kind="ExternalInput")
    bnc = nc.dram_tensor("bnc", [128, 1], f32, kind="ExternalInput")
    wli = nc.dram_tensor("wli", [128, 128], f16, kind="ExternalInput")
    wlj = nc.dram_tensor("wlj", [128, 128], f16, kind="ExternalInput")
    wls = nc.dram_tensor("wls", [S_OUT, 128], f16, kind="ExternalInput")
    idn = nc.dram_tensor("idn", [128, 128], f16, kind="ExternalInput")
    ohb = nc.dram_tensor("ohb", [NWIN, 128, WROWS], f16, kind="ExternalInput")
    Ad = nc.dram_tensor("A", [128, nt, TILE], f16, kind="ExternalInput")
    idxW = nc.dram_tensor("idxW", [ngc, 128, GCH * TILE // 16], i16,
                          kind="ExternalInput")
    out = nc.dram_tensor("out", [128, nt, TILE], f16, kind="ExternalOutput")

    with tile.TileContext(nc) as tc:
        with ExitStack() as ctx:
            dram = ctx.enter_context(
                tc.tile_pool(name="dram", bufs=1, space="DRAM"))
            ci_slice = dram.tile([NLOC, 128], f16)
            ci_tbl = dram.tile([N, 128], f16)

            const = ctx.enter_context(tc.tile_pool(name="const", bufs=1))

            # ---- resident consts (one DMA each via rearrange) ----
            wn_sb = const.tile([128, KC_H, 128], f16)
            nc.sync.dma_start(out=wn_sb[:],
                              in_=wnA.ap().rearrange("(a p) n -> p a n",
                                                     p=128))
            wli_sb = const.tile([128, 128], f16)
            nc.sync.dma_start(out=wli_sb[:], in_=wli.ap())
            wlj_sb = const.tile([128, 128], f16)
            nc.sync.dma_start(out=wlj_sb[:], in_=wlj.ap())
            idn_sb = const.tile([128, 128], f16)
            nc.sync.dma_start(out=idn_sb[:], in_=idn.ap())
            bn_sb = const.tile([128, 1], f32)
            nc.sync.dma_start(out=bn_sb[:], in_=bnc.ap())
            cond_sb = const.tile([128, KC_GB, 128], f16)
            nc.sync.dma_start(out=cond_sb[:],
                              in_=condA.ap().rearrange("(a p) n -> p a n",
                                                       p=128))
            wc_sb = const.tile([128, KC_GB, 256], f16)
            nc.sync.dma_start(out=wc_sb[:],
                              in_=wcA.ap().rearrange("(a p) n -> p a n",
                                                     p=128))
            ohb_sb = const.tile([128, NWIN, WROWS], f16)
            nc.sync.dma_start(out=ohb_sb[:],
                              in_=ohb.ap().rearrange("a p n -> p a n"))
            idx_sb = const.tile([128, ngc, GCH * TILE // 16], i16)
            nc.sync.dma_start(out=idx_sb[:],
                              in_=idxW.ap().rearrange("a p n -> p a n"))
            eps_sb = const.tile([128, 1], f32)
            nc.vector.memset(eps_sb[:], EPS)

            # window tables [ [Cj|gamma|beta](98) ; [Wls|0|0](30) ]
            Wwin = const.tile([128, NWIN, 384], f16)
            nc.vector.memset(Wwin[:], 0.0)
            for blk in range(NWIN):
                nc.sync.dma_start(
                    out=Wwin[WROWS:WROWS + S_OUT, blk, 0:128],
                    in_=wls.ap())

            # ================= phase 1 =================
            with ExitStack() as p1:
                w1 = p1.enter_context(tc.tile_pool(name="w1", bufs=2))
                ps1 = p1.enter_context(
                    tc.tile_pool(name="ps1", bufs=1, space="PSUM"))

                # ht = (nf_c @ Wn)^T : [128f, 800n], four 200-col quarters
                nf_sb = w1.tile([128, KC_H, NLOC], f16, tag="nfsb", bufs=1)
                QTR = NLOC // 4
                for h in range(4):
                    nc.sync.dma_start(
                        out=nf_sb[:, :, h * QTR:(h + 1) * QTR],
                        in_=nfT.ap()[:, h * QTR:(h + 1) * QTR].rearrange(
                            "(a p) n -> p a n", p=128))
                ht_sb = w1.tile([128, NLOC], f16, tag="htsb", bufs=1)
                for h in range(4):
                    ht_ps = ps1.tile([128, QTR], f32, tag="htps", bufs=2)
                    for k in range(KC_H):
                        nc.tensor.matmul(
                            out=ht_ps[:],
                            lhsT=wn_sb[:, k, :],
                            rhs=nf_sb[:, k, h * QTR:(h + 1) * QTR],
                            start=(k == 0), stop=(k == KC_H - 1))
                    nc.scalar.activation(
                        ht_sb[:, h * QTR:(h + 1) * QTR], ht_ps[:],
                        AF.Relu, bias=bn_sb[:])

                # Ci slices first so the AllGather starts ASAP
                for blk in range(NWIN):
                    w0 = blk * WROWS
                    wsz = min(WROWS, NLOC - w0)
                    lhs = ht_sb[:, w0:w0 + wsz]
                    ci_ps = ps1.tile([128, 128], f32, tag="cips", bufs=2)
                    nc.tensor.matmul(out=ci_ps[:wsz, :], lhsT=lhs,
                                     rhs=wli_sb[:], start=True, stop=True)
                    ci_sb = w1.tile([128, 128], f16, tag="cisb", bufs=2)
                    nc.scalar.copy(ci_sb[:wsz, :], ci_ps[:wsz, :])
                    nc.sync.dma_start(
                        out=ci_slice[w0:w0 + wsz, :], in_=ci_sb[:wsz, :])

                # all-gather Ci slices -> full table (overlaps the window
                # table builds below)
                nc.gpsimd.collective_compute(
                    "AllGather", mybir.AluOpType.bypass,
                    replica_groups=[list(range(NCORES))],
                    ins=[ci_slice.opt()],
                    outs=[ci_tbl.opt()])

                # gb = condA.T @ wcA -> [B, 256] f16 (gamma has +1 folded)
                gb_ps = ps1.tile([128, 256], f32, tag="gbps", bufs=1)
                for k in range(KC_GB):
                    nc.tensor.matmul(
                        out=gb_ps[:],
                        lhsT=cond_sb[:, k, :],
                        rhs=wc_sb[:, k, :],
                        start=(k == 0), stop=(k == KC_GB - 1))
                gb_sb = const.tile([128, 256], f16)
                nc.scalar.copy(gb_sb[:], gb_ps[:])

                # per window: [Cj|gamma|beta] -> Wwin
                for blk in range(NWIN):
                    w0 = blk * WROWS
                    wsz = min(WROWS, NLOC - w0)
                    lhs = ht_sb[:, w0:w0 + wsz]
                    cj_ps = ps1.tile([128, 128], f32, tag="cjps", bufs=2)
                    nc.tensor.matmul(out=cj_ps[:wsz, :], lhsT=lhs,
                                     rhs=wlj_sb[:], start=True, stop=True)
                    nc.vector.tensor_copy(Wwin[:wsz, blk, 0:128],
                                          cj_ps[:wsz, :])
                    gbn_ps = ps1.tile([128, 256], f32, tag="gbnps", bufs=1)
                    nc.tensor.matmul(out=gbn_ps[:wsz, :],
                                     lhsT=ohb_sb[:, blk, 0:wsz],
                                     rhs=gb_sb[:], start=True, stop=True)
                    nc.vector.tensor_copy(Wwin[:wsz, blk, 128:384],
                                          gbn_ps[:wsz, :])

            # ================= phase 2 =================
            with ExitStack() as p2:
                gpool = p2.enter_context(tc.tile_pool(name="gp", bufs=5))
                ohp = p2.enter_context(tc.tile_pool(name="ohp", bufs=3))
                stg = p2.enter_context(tc.tile_pool(name="stg", bufs=2))
                xps = p2.enter_context(
                    tc.tile_pool(name="xps", bufs=2, space="PSUM"))

                nch = nt // TPC
                for ch in range(nch):
                    ci_ch = gpool.tile([128, GCH, 128], f16, tag="cich")
                    nc.gpsimd.dma_gather(
                        ci_ch[:], ci_tbl[:],
                        idx_sb[:, ch, :], GCH * TILE, GCH * TILE, 128,
                        single_packet=False)
                    oh = ohp.tile([128, TPC, 128], f16, tag="oh")
                    nc.sync.dma_start(
                        out=oh[:],
                        in_=Ad.ap()[:, ch * TPC:(ch + 1) * TPC, :])
                    st6 = stg.tile([128, TPC, 6], f32, tag="st6")
                    xn = stg.tile([128, TPC, 128], f16, tag="xn")
                    y1 = stg.tile([128, TPC, 128], f16, tag="y1")
                    ob = stg.tile([128, TPC, 128], f16, tag="ob")
                    rstd = stg.tile([128, TPC // 8, 8], f32, tag="rstd")
                    nmr = stg.tile([128, TPC // 8, 8], f32, tag="nmr")

                    Xg = [None] * (TPC // TPG)
                    for q in range(TPC // TPG):
                        X = xps.tile([128, TPG, 512], f32, tag="X")
                        Xg[q] = X
                        for j in range(TPG):
                            t = ch * TPC + q * TPG + j
                            w = sched[t]
                            tt = q * TPG + j
                            nc.tensor.matmul(
                                out=X[:, j, 0:384],
                                lhsT=oh[:, tt, :],
                                rhs=Wwin[:, w, :],
                                start=True, stop=False,
                                skip_group_check=True)
                        nc.tensor.matmul(
                            out=X[:, :, 0:128],
                            lhsT=idn_sb[:],
                            rhs=ci_ch[:, q * TPG:(q + 1) * TPG, :],
                            start=False, stop=True,
                            skip_group_check=True)
                        for j in range(TPG):
                            nc.vector.bn_stats(
                                out=st6[:, q * TPG + j, :],
                                in_=X[:, j, 0:128])

                        if q % 2 == 1:
                            # stats -> rstd/nmr for the last 8 tiles
                            h8 = q // 2
                            s8 = slice((q - 1) * TPG, (q + 1) * TPG)
                            me = st6[:, s8, 1:2]
                            mo = st6[:, s8, 4:5]
                            ve = st6[:, s8, 2:3]
                            vo = st6[:, s8, 5:6]
                            m2x = stg.tile([128, 8], f32, tag="m2x")
                            nc.vector.tensor_tensor(
                                out=m2x[:], in0=me, in1=mo, op=OP.add)
                            dl = stg.tile([128, 8], f32, tag="dl")
                            nc.vector.tensor_tensor(
                                out=dl[:], in0=me, in1=mo, op=OP.subtract)
                            m2p = stg.tile([128, 8], f32, tag="m2p")
                            nc.vector.tensor_tensor(
                                out=m2p[:], in0=ve, in1=vo, op=OP.add)
                            nc.vector.scalar_tensor_tensor(
                                out=dl[:], in0=dl[:], scalar=32.0,
                                in1=dl[:], op0=OP.mult, op1=OP.mult)
                            nc.vector.tensor_tensor(
                                out=m2p[:], in0=m2p[:], in1=dl[:], op=OP.add)
                            # rstd = 1/sqrt(M2/128 + eps)
                            nc.scalar.activation(
                                m2p[:], m2p[:], AF.Sqrt,
                                bias=eps_sb[:], scale=1.0 / 128.0)
                            nc.vector.reciprocal(rstd[:, h8, :], m2p[:])
                            # nmr = -mean*rstd = (m2x * -0.5) * rstd
                            nc.vector.scalar_tensor_tensor(
                                out=nmr[:, h8, :], in0=m2x[:], scalar=-0.5,
                                in1=rstd[:, h8, :], op0=OP.mult, op1=OP.mult)
                            for j8 in range(8):
                                tt = (q - 1) * TPG + j8
                                Xq = Xg[q - 1 + j8 // TPG]
                                nc.scalar.activation(
                                    xn[:, tt, :], Xq[:, j8 % TPG, 0:128],
                                    AF.Identity,
                                    bias=nmr[:, h8, j8:j8 + 1],
                                    scale=rstd[:, h8, j8:j8 + 1])
                            for qq in (q - 1, q):
                                sq = slice(qq * TPG, (qq + 1) * TPG)
                                nc.vector.tensor_tensor(
                                    out=y1[:, sq, :], in0=xn[:, sq, :],
                                    in1=Xg[qq][:, :, 128:256], op=OP.mult)
                                nc.vector.tensor_tensor(
                                    out=ob[:, sq, :], in0=y1[:, sq, :],
                                    in1=Xg[qq][:, :, 256:384], op=OP.add)
                    nc.scalar.activation(ob[:], ob[:], AF.Relu)
                    nc.sync.dma_start(
                        out=out.ap()[:, ch * TPC:(ch + 1) * TPC, :],
                        in_=ob[:])

    nc.compile()
    return nc


def _run(inputs, trace=False):
    _axon_shim()
    from concourse.bass_utils import run_bass_kernel_spmd

    jj = np.asarray(inputs["node_j_ids"])
    key = hash(jj.tobytes())
    if _cache.get("key") != key:
        plans, sched = _plan(inputs)
        _cache.update(
            key=key, plans=plans, sched=sched, nc=_build_program(sched))
    plans, sched, nc = _cache["plans"], _cache["sched"], _cache["nc"]
    in_maps = _prep_inputs(inputs, plans, sched)

    res = run_bass_kernel_spmd(
        nc, in_maps, core_ids=list(range(NCORES)), trace=trace
    )
    full = np.zeros((E, 128), np.float32)
    for c, p in enumerate(plans):
        sm_ = p["slotmap"]          # [slot(part), tile]
        valid = sm_ >= 0
        full[sm_[valid]] = res.results[c]["out"][valid].astype(np.float32)
    return full, res


def kernel(**inputs):
    full, _ = _run(inputs, trace=False)
    return full.astype(np.float32)
